# revision 24
# baseline (speedup 1.0000x reference)
"""KMeans vq_codebook step on 8 NeuronCores (Trainium2, Bass/Tile), fp8 DoubleRow.

Data-parallel over N: each core gets an x shard [8192, 512] pre-quantized to
fp8e4m3 AND pre-transposed on the host (xT[p, t, j, n] = x8[t*128+n, j*128+p]),
centers replicated. Per core, per 128-point tile:
  xT8  = DMA tile [128, DC, 128] fp8 (contiguous 512B lines)
  ps   = fp8 DoubleRow seed (-||c||^2 3-term split) + fp8 DoubleRow 2*x@c.T,
         both K-halves interleaved so weight loads overlap streams
  m8   = DVE InstMax top-8 read directly from PSUM
  mask = ACT Sign(m - s) in fp8: 0 at argmax, 1 elsewhere
  hist[c,k] += onehot(y).T @ mask  every 2 tiles via fp8 DoubleRow pairs
Host: loss = sum(x^2) - sum(m); counts = classcount - hist; acc from counts.
"""
import sys

sys.path.insert(0, "/opt/trn_rl_repo")

import numpy as np
import ml_dtypes

import concourse.bass as bass
import concourse.mybir as mybir
from concourse import bacc
from concourse.bass import ds, ts
from concourse.bass_utils import run_bass_kernel_spmd
from concourse.tile import TileContext

dt = mybir.dt
F32 = dt.float32
BF16 = dt.bfloat16
FP8 = dt.float8e4
AF = mybir.ActivationFunctionType
ALU = mybir.AluOpType
DR = mybir.MatmulPerfMode.DoubleRow
F8NP = ml_dtypes.float8_e4m3
BF16NP = ml_dtypes.bfloat16

N, D, K, NCLS, NCORES = 65536, 512, 1024, 10, 8
NSH = N // NCORES          # 8192 points per core
PT = NSH // 128            # 64 point-tiles per core
DC = D // 128              # 4 contraction chunks (2 DoubleRow pairs)
KH = K // 512              # 2 free-dim halves
NC16 = 16                  # one-hot padded to 16 classes


def _build():
    nc = bacc.Bacc(None, target_bir_lowering=False, debug=False)
    xT_in = nc.dram_tensor("xT8", [128, PT, DC, 128], FP8, kind="ExternalInput")
    cT8_in = nc.dram_tensor("cT8", [128, DC, K], FP8, kind="ExternalInput")
    aug_in = nc.dram_tensor("aug", [2, 2, K], FP8, kind="ExternalInput")
    cf_in = nc.dram_tensor("cf", [2, 2, 128], FP8, kind="ExternalInput")
    oh_in = nc.dram_tensor("ohp", [128, PT // 2, 2, NC16], FP8,
                           kind="ExternalInput")
    counts_out = nc.dram_tensor("counts", [NC16, K], F32, kind="ExternalOutput")
    lossm_out = nc.dram_tensor("lossm", [128, 1], F32, kind="ExternalOutput")

    with TileContext(nc) as tc:
        with (
            tc.tile_pool(name="persist", bufs=1) as pp,
            tc.tile_pool(name="work", bufs=3) as wp,
            tc.tile_pool(name="psA", bufs=2, space="PSUM") as psA,   # s tiles
            tc.tile_pool(name="psH", bufs=1, space="PSUM") as psH,   # histogram
            tc.tile_pool(name="psW", bufs=1, space="PSUM") as psW,   # warm/ka
        ):
            cT8 = pp.tile([128, DC, K], FP8)
            nc.sync.dma_start(out=cT8[:], in_=cT8_in[:, :, :])
            aug = pp.tile([2, 2, K], FP8)
            nc.sync.dma_start(out=aug[:], in_=aug_in[:, :, :])
            cf8 = pp.tile([2, 2, 128], FP8)
            nc.sync.dma_start(out=cf8[:], in_=cf_in[:, :, :])
            ohp = pp.tile([128, PT // 2, 2, NC16], FP8)
            nc.sync.dma_start(out=ohp[:], in_=oh_in[:, :, :, :])

            m8buf = pp.tile([128, PT * 8], F32)
            hist = psH.tile([NC16, K], F32)

            # PE warmup: dedicated non-cycling psum buffer, back-to-back
            # matmuls so the HAM clock-gate opens (cold PE runs at half clock).
            wt_f = pp.tile([128, 128], F32)
            nc.vector.memset(wt_f[:], 0.0)
            wt = wt_f[:].bitcast(BF16)[:, 0:128]
            ka_f = pp.tile([128, 512], F32)
            nc.vector.memset(ka_f[:], 0.0)
            kw = wt_f[:].bitcast(dt.float32r)[:, 0:128]
            ka = ka_f[:].bitcast(dt.float32r)[:, 0:512]
            wps = psW.tile([128, 512], F32)
            for _ in range(40):
                nc.tensor.matmul(wps[:, 0:128], wt, wt, start=True, stop=True,
                                 skip_group_check=True)

            def hist_pair(p, mpt):
                # hist matmuls for mask pair p, emitted one tile late so the
                # in-order PE queue never stalls on the Sign producing the mask
                for kh in range(KH):
                    nc.tensor.matmul(hist[:, ds(kh * 512, 512)],
                                     ohp[:, p, :, :],
                                     mpt[:, :, ds(kh * 512, 512)],
                                     start=(p == 0), stop=(p == PT // 2 - 1),
                                     perf_mode=DR, skip_group_check=True)

            mh = pp.tile([128, 16], F32)     # per-half max8 scratch
            mp = None
            mp_prev = None
            for t in range(PT):
                xT8 = wp.tile([128, DC, 128], FP8, tag="xT8")
                nc.sync.dma_start(out=xT8[:], in_=xT_in[:, t, :, :])

                # f32r keepalive: dependency-free, fills PE idle gaps so the
                # HAM clock-gate stays open (fp8-only streams don't hold it)
                if t % 2 == 0:
                    nc.tensor.matmul(wps[:, 0:512], kw, ka, start=True,
                                     stop=True, skip_group_check=True)
                ps = psA.tile([128, K], F32, tag="ps")
                # kh-half 0 fully first, so its MAX8 half-scan on DVE runs
                # while PE still streams kh-half 1 (shortens the ps critical
                # path: seeds+mains -> max -> sign -> buffer release)
                for kh in range(KH):
                    nc.tensor.matmul(ps[:, ds(kh * 512, 512)], cf8[:],
                                     aug[:, :, ds(kh * 512, 512)],
                                     start=True, stop=False,
                                     perf_mode=DR, skip_group_check=True)
                    for j in range(DC // 2):
                        nc.tensor.matmul(ps[:, ds(kh * 512, 512)],
                                         xT8[:, ds(2 * j, 2), :],
                                         cT8[:, ds(2 * j, 2), ds(kh * 512, 512)],
                                         start=False, stop=(j == DC // 2 - 1),
                                         perf_mode=DR, skip_group_check=True)
                    nc.vector.max(mh[:, ts(kh, 8)], ps[:, ds(kh * 512, 512)])
                if t % 2 == 1 and t >= 3:
                    hist_pair(t // 2 - 1, mp_prev)

                nc.vector.tensor_tensor(out=m8buf[:, t * 8:t * 8 + 1],
                                        in0=mh[:, 0:1], in1=mh[:, 8:9],
                                        op=ALU.max)
                if t % 2 == 0:
                    mp_prev = mp
                    mp = wp.tile([128, 2, K], FP8, tag="mask")
                nc.scalar.activation(mp[:, t % 2, :], ps[:], AF.Sign,
                                     bias=m8buf[:, t * 8:t * 8 + 1], scale=-1.0)
            hist_pair(PT // 2 - 1, mp)

            # tail: loss partials + counts to DRAM
            lossb = pp.tile([128, 1], F32)
            m8v = m8buf[:].rearrange("p (t e) -> p t e", e=8)[:, :, 0:1]
            nc.vector.tensor_reduce(lossb[:, 0:1], m8v, axis=mybir.AxisListType.XY,
                                    op=ALU.add)
            nc.sync.dma_start(out=lossm_out[:], in_=lossb[:])
            csb = pp.tile([NC16, K], F32)
            nc.scalar.copy(csb[:], hist[:])
            nc.sync.dma_start(out=counts_out[:], in_=csb[:])

    nc.finalize()
    return nc


_NC_CACHE: dict = {}


def _get_nc():
    if "nc" not in _NC_CACHE:
        _NC_CACHE["nc"] = _build()
    return _NC_CACHE["nc"]


def _prep_host(x, centers, y):
    x = np.ascontiguousarray(np.asarray(x, dtype=np.float32))
    centers = np.ascontiguousarray(np.asarray(centers, dtype=np.float32))
    y = np.ascontiguousarray(np.asarray(y, dtype=np.int32))

    x8 = x.astype(F8NP)                                   # [N, D]
    x2sum = float(np.dot(x.reshape(-1), x.reshape(-1)))   # scalar, f32 blas

    # per-core transposed tile-major layout: xT[p, t, j, n] = x8[t*128+n, j*128+p]
    xT_all = []
    for cid in range(NCORES):
        xs = x8[cid * NSH:(cid + 1) * NSH]                # [NSH, D]
        xT = xs.reshape(PT, 128, DC, 128).transpose(3, 0, 2, 1)
        xT_all.append(np.ascontiguousarray(xT))           # [128, PT, DC, 128]

    c2 = (centers.astype(np.float64) ** 2).sum(1).astype(np.float32)  # [K]
    cT = np.ascontiguousarray((2.0 * centers).astype(F8NP).T)         # [D, K]
    cT8 = np.ascontiguousarray(
        cT.reshape(DC, 128, K).transpose(1, 0, 2))                    # [128, DC, K]

    # -c2 as fp8 DoubleRow seed rows: -c2 = 3*Q(-c2/3) + Q(rem) + Q(rem2)
    r1 = (-c2 / 3.0).astype(F8NP)
    rem = -c2 - 3.0 * r1.astype(np.float32)
    r4 = rem.astype(F8NP)
    rem2 = rem - r4.astype(np.float32)
    r5 = rem2.astype(F8NP)
    aug = np.zeros((2, 2, K), dtype=F8NP)
    aug[0, 0] = r1
    aug[1, 0] = r4
    aug[0, 1] = r5
    cf = np.zeros((2, 2, 128), dtype=F8NP)
    cf[0, 0] = 3.0
    cf[1, 0] = 1.0
    cf[0, 1] = 1.0

    # paired one-hot labels: ohp[p, tp, j, c] = (y[(2tp+j)*128+p] == c), fp8
    ohp_all = []
    for cid in range(NCORES):
        ysh = y[cid * NSH:(cid + 1) * NSH].reshape(PT // 2, 2, 128)   # [tp, j, p]
        oh = (ysh[..., None] == np.arange(NC16)[None, None, None, :])
        ohp_all.append(np.ascontiguousarray(
            oh.transpose(2, 0, 1, 3).astype(F8NP)))       # [128, PT//2, 2, 16]
    classcount = np.bincount(y, minlength=NCLS).astype(np.float64)    # [10]
    return xT_all, cT8, aug, cf, ohp_all, classcount, x2sum


def kernel(x, centers, y, _trace=False):
    xT_all, cT8, aug, cf, ohp_all, classcount, x2sum = _prep_host(x, centers, y)
    nc = _get_nc()
    in_maps = [
        {"xT8": xT_all[c], "cT8": cT8, "aug": aug, "cf": cf, "ohp": ohp_all[c]}
        for c in range(NCORES)
    ]
    res = run_bass_kernel_spmd(nc, in_maps, core_ids=list(range(NCORES)),
                               trace=_trace)
    dev = np.zeros((NC16, K), np.float64)
    msum = 0.0
    for r in res.results:
        dev += r["counts"].astype(np.float64)
        msum += r["lossm"].astype(np.float64).sum()
    loss = x2sum - msum
    counts = classcount[:, None] - dev[:NCLS, :]          # [10, K]
    correct = counts.max(axis=0).sum()
    acc = np.float32(correct / N)
    out = (np.float32(loss), acc)
    if _trace:
        return out, res
    return out


# revision 25
# speedup vs baseline: 1.0031x; 1.0031x over previous
"""KMeans vq_codebook step on 8 NeuronCores (Trainium2, Bass/Tile), fp8 DoubleRow.

Data-parallel over N: each core gets an x shard [8192, 512] pre-quantized to
fp8e4m3 AND pre-transposed on the host (xT[p, t, j, n] = x8[t*128+n, j*128+p]),
centers replicated. Per core, per 128-point tile:
  xT8  = DMA tile [128, DC, 128] fp8 (contiguous 512B lines)
  ps   = fp8 DoubleRow seed (-||c||^2 3-term split) + fp8 DoubleRow 2*x@c.T,
         both K-halves interleaved so weight loads overlap streams
  m8   = DVE InstMax top-8 read directly from PSUM
  mask = ACT Sign(m - s) in fp8: 0 at argmax, 1 elsewhere
  hist[c,k] += onehot(y).T @ mask  every 2 tiles via fp8 DoubleRow pairs
Host: loss = sum(x^2) - sum(m); counts = classcount - hist; acc from counts.
"""
import sys

sys.path.insert(0, "/opt/trn_rl_repo")

import numpy as np
import ml_dtypes

import concourse.bass as bass
import concourse.mybir as mybir
from concourse import bacc
from concourse.bass import ds, ts
from concourse.bass_utils import run_bass_kernel_spmd
from concourse.tile import TileContext

dt = mybir.dt
F32 = dt.float32
BF16 = dt.bfloat16
FP8 = dt.float8e4
AF = mybir.ActivationFunctionType
ALU = mybir.AluOpType
DR = mybir.MatmulPerfMode.DoubleRow
F8NP = ml_dtypes.float8_e4m3
BF16NP = ml_dtypes.bfloat16

N, D, K, NCLS, NCORES = 65536, 512, 1024, 10, 8
NSH = N // NCORES          # 8192 points per core
PT = NSH // 128            # 64 point-tiles per core
DC = D // 128              # 4 contraction chunks (2 DoubleRow pairs)
KH = K // 512              # 2 free-dim halves
NC16 = 16                  # one-hot padded to 16 classes


def _build():
    nc = bacc.Bacc(None, target_bir_lowering=False, debug=False)
    xT_in = nc.dram_tensor("xT8", [128, PT, DC, 128], FP8, kind="ExternalInput")
    cT8_in = nc.dram_tensor("cT8", [128, DC, K], FP8, kind="ExternalInput")
    aug_in = nc.dram_tensor("aug", [2, 2, K], FP8, kind="ExternalInput")
    cf_in = nc.dram_tensor("cf", [2, 2, 128], FP8, kind="ExternalInput")
    oh_in = nc.dram_tensor("ohp", [128, PT // 2, 2, NC16], FP8,
                           kind="ExternalInput")
    counts_out = nc.dram_tensor("counts", [NC16, K], F32, kind="ExternalOutput")
    lossm_out = nc.dram_tensor("lossm", [128, 1], F32, kind="ExternalOutput")

    with TileContext(nc) as tc:
        with (
            tc.tile_pool(name="persist", bufs=1) as pp,
            tc.tile_pool(name="work", bufs=3) as wp,
            tc.tile_pool(name="psA", bufs=2, space="PSUM") as psA,   # s tiles
            tc.tile_pool(name="psH", bufs=1, space="PSUM") as psH,   # histogram
            tc.tile_pool(name="psW", bufs=1, space="PSUM") as psW,   # warm/ka
        ):
            cT8 = pp.tile([128, DC, K], FP8)
            nc.sync.dma_start(out=cT8[:], in_=cT8_in[:, :, :])
            aug = pp.tile([2, 2, K], FP8)
            nc.sync.dma_start(out=aug[:], in_=aug_in[:, :, :])
            cf8 = pp.tile([2, 2, 128], FP8)
            nc.sync.dma_start(out=cf8[:], in_=cf_in[:, :, :])
            ohp = pp.tile([128, PT // 2, 2, NC16], FP8)
            nc.sync.dma_start(out=ohp[:], in_=oh_in[:, :, :, :])

            m8buf = pp.tile([128, PT * 8], F32)
            hist = psH.tile([NC16, K], F32)

            # PE warmup: dedicated non-cycling psum buffer, back-to-back
            # matmuls so the HAM clock-gate opens (cold PE runs at half clock).
            wt_f = pp.tile([128, 128], F32)
            nc.vector.memset(wt_f[:], 0.0)
            wt = wt_f[:].bitcast(BF16)[:, 0:128]
            ka_f = pp.tile([128, 512], F32)
            nc.vector.memset(ka_f[:], 0.0)
            kw = wt_f[:].bitcast(dt.float32r)[:, 0:128]
            ka = ka_f[:].bitcast(dt.float32r)[:, 0:512]
            wps = psW.tile([128, 512], F32)
            for _ in range(40):
                nc.tensor.matmul(wps[:, 0:128], wt, wt, start=True, stop=True,
                                 skip_group_check=True)

            def hist_pair(p, mpt):
                # hist matmuls for mask pair p, emitted one tile late so the
                # in-order PE queue never stalls on the Sign producing the mask
                for kh in range(KH):
                    nc.tensor.matmul(hist[:, ds(kh * 512, 512)],
                                     ohp[:, p, :, :],
                                     mpt[:, :, ds(kh * 512, 512)],
                                     start=(p == 0), stop=(p == PT // 2 - 1),
                                     perf_mode=DR, skip_group_check=True)

            mp = None
            mp_prev = None
            for t in range(PT):
                # f32r keepalive: dependency-free, fills PE idle gaps so the
                # HAM clock-gate stays open until the fp8 stream sustains it
                if t % 2 == 0 and t < PT // 2:
                    nc.tensor.matmul(wps[:, 0:512], kw, ka, start=True,
                                     stop=True, skip_group_check=True)
                xT8 = wp.tile([128, DC, 128], FP8, tag="xT8")
                nc.sync.dma_start(out=xT8[:], in_=xT_in[:, t, :, :])

                ps = psA.tile([128, K], F32, tag="ps")
                # interleave the two K-half chains so each matmul's weight
                # load can overlap the other chain's column stream
                for kh in range(KH):
                    nc.tensor.matmul(ps[:, ds(kh * 512, 512)], cf8[:],
                                     aug[:, :, ds(kh * 512, 512)],
                                     start=True, stop=False,
                                     perf_mode=DR, skip_group_check=True)
                for j in range(DC // 2):
                    for kh in range(KH):
                        nc.tensor.matmul(ps[:, ds(kh * 512, 512)],
                                         xT8[:, ds(2 * j, 2), :],
                                         cT8[:, ds(2 * j, 2), ds(kh * 512, 512)],
                                         start=False, stop=(j == DC // 2 - 1),
                                         perf_mode=DR, skip_group_check=True)
                if t % 2 == 1 and t >= 3:
                    hist_pair(t // 2 - 1, mp_prev)

                nc.vector.max(m8buf[:, ts(t, 8)], ps[:])
                if t % 2 == 0:
                    mp_prev = mp
                    mp = wp.tile([128, 2, K], FP8, tag="mask")
                nc.scalar.activation(mp[:, t % 2, :], ps[:], AF.Sign,
                                     bias=m8buf[:, t * 8:t * 8 + 1], scale=-1.0)
            hist_pair(PT // 2 - 1, mp)

            # tail: loss partials + counts to DRAM
            lossb = pp.tile([128, 1], F32)
            m8v = m8buf[:].rearrange("p (t e) -> p t e", e=8)[:, :, 0:1]
            nc.vector.tensor_reduce(lossb[:, 0:1], m8v, axis=mybir.AxisListType.XY,
                                    op=ALU.add)
            nc.sync.dma_start(out=lossm_out[:], in_=lossb[:])
            csb = pp.tile([NC16, K], F32)
            nc.scalar.copy(csb[:], hist[:])
            nc.sync.dma_start(out=counts_out[:], in_=csb[:])

    nc.finalize()
    return nc


_NC_CACHE: dict = {}


def _get_nc():
    if "nc" not in _NC_CACHE:
        _NC_CACHE["nc"] = _build()
    return _NC_CACHE["nc"]


def _prep_host(x, centers, y):
    x = np.ascontiguousarray(np.asarray(x, dtype=np.float32))
    centers = np.ascontiguousarray(np.asarray(centers, dtype=np.float32))
    y = np.ascontiguousarray(np.asarray(y, dtype=np.int32))

    x8 = x.astype(F8NP)                                   # [N, D]
    x2sum = float(np.dot(x.reshape(-1), x.reshape(-1)))   # scalar, f32 blas

    # per-core transposed tile-major layout: xT[p, t, j, n] = x8[t*128+n, j*128+p]
    xT_all = []
    for cid in range(NCORES):
        xs = x8[cid * NSH:(cid + 1) * NSH]                # [NSH, D]
        xT = xs.reshape(PT, 128, DC, 128).transpose(3, 0, 2, 1)
        xT_all.append(np.ascontiguousarray(xT))           # [128, PT, DC, 128]

    c2 = (centers.astype(np.float64) ** 2).sum(1).astype(np.float32)  # [K]
    cT = np.ascontiguousarray((2.0 * centers).astype(F8NP).T)         # [D, K]
    cT8 = np.ascontiguousarray(
        cT.reshape(DC, 128, K).transpose(1, 0, 2))                    # [128, DC, K]

    # -c2 as fp8 DoubleRow seed rows: -c2 = 3*Q(-c2/3) + Q(rem) + Q(rem2)
    r1 = (-c2 / 3.0).astype(F8NP)
    rem = -c2 - 3.0 * r1.astype(np.float32)
    r4 = rem.astype(F8NP)
    rem2 = rem - r4.astype(np.float32)
    r5 = rem2.astype(F8NP)
    aug = np.zeros((2, 2, K), dtype=F8NP)
    aug[0, 0] = r1
    aug[1, 0] = r4
    aug[0, 1] = r5
    cf = np.zeros((2, 2, 128), dtype=F8NP)
    cf[0, 0] = 3.0
    cf[1, 0] = 1.0
    cf[0, 1] = 1.0

    # paired one-hot labels: ohp[p, tp, j, c] = (y[(2tp+j)*128+p] == c), fp8
    ohp_all = []
    for cid in range(NCORES):
        ysh = y[cid * NSH:(cid + 1) * NSH].reshape(PT // 2, 2, 128)   # [tp, j, p]
        oh = (ysh[..., None] == np.arange(NC16)[None, None, None, :])
        ohp_all.append(np.ascontiguousarray(
            oh.transpose(2, 0, 1, 3).astype(F8NP)))       # [128, PT//2, 2, 16]
    classcount = np.bincount(y, minlength=NCLS).astype(np.float64)    # [10]
    return xT_all, cT8, aug, cf, ohp_all, classcount, x2sum


def kernel(x, centers, y, _trace=False):
    xT_all, cT8, aug, cf, ohp_all, classcount, x2sum = _prep_host(x, centers, y)
    nc = _get_nc()
    in_maps = [
        {"xT8": xT_all[c], "cT8": cT8, "aug": aug, "cf": cf, "ohp": ohp_all[c]}
        for c in range(NCORES)
    ]
    res = run_bass_kernel_spmd(nc, in_maps, core_ids=list(range(NCORES)),
                               trace=_trace)
    dev = np.zeros((NC16, K), np.float64)
    msum = 0.0
    for r in res.results:
        dev += r["counts"].astype(np.float64)
        msum += r["lossm"].astype(np.float64).sum()
    loss = x2sum - msum
    counts = classcount[:, None] - dev[:NCLS, :]          # [10, K]
    correct = counts.max(axis=0).sum()
    acc = np.float32(correct / N)
    out = (np.float32(loss), acc)
    if _trace:
        return out, res
    return out
